# revision 17
# baseline (speedup 1.0000x reference)
"""Trainium2 Bass kernel for a dense transformer decoder layer (fp32 I/O).

Model: B=4, T=2048, H=16 heads, DH=64, D=1024, DFF=4096.
  qkv = x @ w_qkv + b_qkv ; non-causal attention (mask==1) ; residual+LN1 ;
  relu FFN (D->DFF->D) ; residual+LN2.

Sharding over 8 NeuronCores: core c handles batch b=c//2 and head-group
hg=c%2 (8 of 16 heads) for QKV+attention over the full sequence; the core
pair (2b, 2b+1) exchanges attention-output halves with a pairwise
AllReduce(add), and each core runs LN1/FFN/LN2 for its 1024-token half.

v2 design notes (vs the f32r baseline):
  - Host pre-transposes x and pre-packs all weights in bf16; all matmuls
    are bf16 x bf16 (separate, overlappable LDWEIGHTS; f32r is
    self-loading so every MM paid its weight load serially).
  - Attention processes head pairs with row-tiled concurrent matmuls
    (tile_position (0,0)/(64,0)) for the K=64 logits contraction.
  - Softmax exp is split across engines: even head on ACT (Exp, bf16
    out), odd head on DVE via a 1-op Schraudolph in bf16 bit-space
    (tensor_scalar f32->int16, reinterpret as bf16).
  - Softmax denominators ride as a ones-row in the PV stationary; the
    reciprocal uses reciprocal_approx_fast (DVE InstReciprocal is 3.3us).
  - The pairwise exchange is AllReduce(add) on the partner-token half;
    my-half y and my-sent stay in SBUF (no DRAM round trip).
  - FFN weights are loaded once (token-loop inside), h kept in bf16,
    relu+bias on ACT.
"""
import os
import sys
import types

import numpy as np
import ml_dtypes

if "/opt/trn_rl_repo" not in sys.path:
    sys.path.insert(0, "/opt/trn_rl_repo")

BF16NP = ml_dtypes.bfloat16

B, T, H, DH = 4, 2048, 16, 64
D = H * DH            # 1024
DFF = 4096
LN_EPS = 1e-5
N_CORES = 8
TLOC = T // 2         # tokens per core in the FFN phase
HLOC = H // 2         # heads per core

A16 = 184.6650        # 2^7 / ln 2 (bf16 bit-space Schraudolph)
B16 = 16250.0

_PROGRAM = None


def _install_ntff_hook():
    try:
        import antenv
        if "antenv.axon_hooks" in sys.modules:
            return
        mod = types.ModuleType("antenv.axon_hooks")
        holder = [None]
        mod.set_axon_ntff_profile_hook = lambda h: holder.__setitem__(0, h)
        mod.get_axon_ntff_profile_hook = lambda: holder[0]
        sys.modules["antenv.axon_hooks"] = mod
        antenv.axon_hooks = mod
        from trn_agent_boot.trn_boot import _ntff_profile_via_ctypes
        mod.set_axon_ntff_profile_hook(
            _ntff_profile_via_ctypes("/opt/axon/libaxon_pjrt.so"))
    except Exception:
        pass


def _build_program():
    import concourse.bass as bass
    import concourse.mybir as mybir
    import concourse.tile as tile
    from concourse import bacc

    F32 = mybir.dt.float32
    BF = mybir.dt.bfloat16
    I16 = mybir.dt.int16
    AF = mybir.ActivationFunctionType
    ALU = mybir.AluOpType

    nc = bacc.Bacc("TRN2", target_bir_lowering=False, debug=False,
                   num_devices=N_CORES)

    xT_d = nc.dram_tensor("xT", [D, TLOC], F32, kind="ExternalInput").ap()
    xb_d = nc.dram_tensor("xb", [D, T], BF, kind="ExternalInput").ap()
    wq_d = nc.dram_tensor("wq", [D, 512], BF, kind="ExternalInput").ap()
    wk_d = nc.dram_tensor("wk", [D, 512], BF, kind="ExternalInput").ap()
    wv_d = nc.dram_tensor("wv", [D, 512], BF, kind="ExternalInput").ap()
    bq_d = nc.dram_tensor("bq", [128, 4], F32, kind="ExternalInput").ap()
    bk_d = nc.dram_tensor("bk", [128, 4], F32, kind="ExternalInput").ap()
    bv_d = nc.dram_tensor("bv", [128, 8], F32, kind="ExternalInput").ap()
    wff_d = nc.dram_tensor("wff", [D, DFF], BF, kind="ExternalInput").ap()
    bff_d = nc.dram_tensor("bff", [128, 32], F32, kind="ExternalInput").ap()
    wout_d = nc.dram_tensor("wout", [DFF, D], BF, kind="ExternalInput").ap()
    bout_d = nc.dram_tensor("bout", [128, 8], F32, kind="ExternalInput").ap()
    lnw1_d = nc.dram_tensor("lnw1", [128, 8], F32, kind="ExternalInput").ap()
    lnb1_d = nc.dram_tensor("lnb1", [128, 8], F32, kind="ExternalInput").ap()
    lnw2_d = nc.dram_tensor("lnw2", [128, 8], F32, kind="ExternalInput").ap()
    lnb2_d = nc.dram_tensor("lnb2", [128, 8], F32, kind="ExternalInput").ap()
    out_d = nc.dram_tensor("outT", [D, TLOC], F32, kind="ExternalOutput").ap()

    with tile.TileContext(nc) as tc:
        constp = tc.alloc_tile_pool(name="const", bufs=1)
        dramp = tc.alloc_tile_pool(name="dram", bufs=1, space="DRAM")

        eps128 = constp.tile([128, 1], F32)
        nc.vector.memset(eps128[:], LN_EPS)
        ones_mat = constp.tile([128, 128], BF)
        nc.vector.memset(ones_mat[:].bitcast(mybir.dt.uint16), 0x3F80)
        ones_c64 = constp.tile([1, 64], BF)
        nc.vector.memset(ones_c64[:].bitcast(mybir.dt.uint16), 0x3F80)

        bias_tiles = {}
        for name, d_ap, w in [("bq", bq_d, 4), ("bk", bk_d, 4), ("bv", bv_d, 8),
                              ("bff", bff_d, 32), ("bout", bout_d, 8),
                              ("lnw1", lnw1_d, 8), ("lnb1", lnb1_d, 8),
                              ("lnw2", lnw2_d, 8), ("lnb2", lnb2_d, 8)]:
            t = constp.tile([128, w], F32, tag=name)
            nc.sync.dma_start(out=t[:], in_=d_ap)
            bias_tiles[name] = t
        bq_sb, bk_sb, bv_sb = bias_tiles["bq"], bias_tiles["bk"], bias_tiles["bv"]
        bff_sb, bout_sb = bias_tiles["bff"], bias_tiles["bout"]
        lnw1_sb, lnb1_sb = bias_tiles["lnw1"], bias_tiles["lnb1"]
        lnw2_sb, lnb2_sb = bias_tiles["lnw2"], bias_tiles["lnb2"]

        # ============ persistent SBUF state ============
        pXM = tc.alloc_tile_pool(name="pXM", bufs=1)
        xm = [pXM.tile([128, TLOC], F32, tag=f"xm{d}", name=f"xm{d}")
              for d in range(8)]
        pXSB = tc.alloc_tile_pool(name="pXSB", bufs=1)
        xsb = [pXSB.tile([128, TLOC], BF, tag=f"xsb{d}", name=f"xsb{d}")
               for d in range(8)]
        pY = tc.alloc_tile_pool(name="pY", bufs=1)
        ymine = [pY.tile([128, TLOC], F32, tag=f"ym{i}", name=f"ym{i}")
                 for i in range(4)]
        ysent = [pY.tile([128, TLOC], F32, tag=f"ys{i}", name=f"ys{i}")
                 for i in range(4)]
        pQKV = tc.alloc_tile_pool(name="pQKV", bufs=1)
        qT = [pQKV.tile([128, T], BF, tag=f"qT{i}", name=f"qT{i}")
              for i in range(4)]
        kT = [pQKV.tile([128, T], BF, tag=f"kT{i}", name=f"kT{i}")
              for i in range(4)]
        v_sb = [pQKV.tile([128, 8, 65], BF, tag=f"v{i}", name=f"v{i}")
                for i in range(16)]
        cc_in = dramp.tile([512, TLOC], F32, tag="ccin", name="ccin")
        cc_out = dramp.tile([512, TLOC], F32, tag="ccout", name="ccout")

        # ================= Phase B: QKV projections =================
        with tc.tile_pool(name="xbp", bufs=1) as xbp, \
             tc.tile_pool(name="wqk", bufs=1) as wqkp, \
             tc.tile_pool(name="psQK", bufs=6, space="PSUM") as psQK, \
             tc.tile_pool(name="psV", bufs=2, space="PSUM") as psV:
            xb = [xbp.tile([128, T], BF, tag=f"xb{d}", name=f"xb{d}")
                  for d in range(8)]
            for d in range(8):
                nc.sync.dma_start(out=xb[d][:],
                                  in_=xb_d[d * 128:(d + 1) * 128, :])
            with nc.named_scope("phB_qkv"):
                # q/k projections: stationary w chunk, moving xb
                wq_sb = [wqkp.tile([128, 512], BF, tag=f"wq{d}", name=f"wq{d}") for d in range(8)]
                wk_sb = [wqkp.tile([128, 512], BF, tag=f"wk{d}", name=f"wk{d}") for d in range(8)]
                wv_sb = [wqkp.tile([128, 512], BF, tag=f"wv{d}", name=f"wv{d}") for d in range(8)]
                for d in range(8):
                    nc.sync.dma_start(out=wq_sb[d][:],
                                      in_=wq_d[d * 128:(d + 1) * 128, :])
                    nc.sync.dma_start(out=wk_sb[d][:],
                                      in_=wk_d[d * 128:(d + 1) * 128, :])
                    nc.sync.dma_start(out=wv_sb[d][:],
                                      in_=wv_d[d * 128:(d + 1) * 128, :])
                for ct in range(4):
                    for (w_sb, b_sb, dst) in [(wk_sb, bk_sb, kT),
                                              (wq_sb, bq_sb, qT)]:
                        pss = [psQK.tile([128, 512], F32, tag="qk", name="qk")
                               for _ in range(4)]
                        for d in range(8):
                            for tb in range(4):
                                nc.tensor.matmul(
                                    pss[tb][:],
                                    w_sb[d][:, ct * 128:(ct + 1) * 128],
                                    xb[d][:, tb * 512:(tb + 1) * 512],
                                    start=(d == 0), stop=(d == 7))
                        for tb in range(4):
                            nc.vector.tensor_scalar_add(
                                dst[ct][:, tb * 512:(tb + 1) * 512],
                                pss[tb][:], b_sb[:, ct:ct + 1])
                # v projection: stationary xb chunk, moving wv
                for tt in range(16):
                    ps = psV.tile([128, 512], F32, tag="v", name="v")
                    for d in range(8):
                        nc.tensor.matmul(
                            ps[:], xb[d][:, tt * 128:(tt + 1) * 128],
                            wv_sb[d][:], start=(d == 0), stop=(d == 7))
                    nc.vector.tensor_copy(
                        v_sb[tt][:, :, 0:64],
                        ps[:].rearrange("p (h e) -> p h e", h=8))
                    nc.vector.memset(
                        v_sb[tt][:, :, 64:65].bitcast(mybir.dt.uint16), 0x3F80)

        # ================= Phase C: attention =================
        for d in range(8):
            nc.sync.dma_start(out=xm[d][:], in_=xT_d[d * 128:(d + 1) * 128, :])
        with tc.tile_pool(name="psL", bufs=2, space="PSUM") as psLp, \
             tc.tile_pool(name="psPV", bufs=2, space="PSUM") as psPVp, \
             tc.tile_pool(name="PT", bufs=2) as PTp, \
             tc.tile_pool(name="nrm", bufs=2) as nrmp:
            with nc.named_scope("phC_attn"):
                def normalize(hp, tb, pvs_e, pvs_o):
                    he, ho = 2 * hp, 2 * hp + 1
                    for hh, pvs in ((he, pvs_e), (ho, pvs_o)):
                        dbr = nrmp.tile([1, 512], BF, tag="dbr", name="dbr")
                        nc.scalar.copy(dbr[:], pvs[64:65, :])
                        bcp = psLp.tile([128, 512], F32, tag="Le",
                                        name="bcp")
                        nc.tensor.matmul(bcp[0:64, :], ones_c64[:],
                                         dbr[:], start=True, stop=True)
                        rec = nrmp.tile([64, 512], F32, tag="rec", name="rec")
                        nc.vector.reciprocal_approx_fast(
                            out=rec[:], in_=bcp[0:64, :])
                        row = (hh // 2)
                        half = (hh % 2) * 64
                        ydst = ymine if tb < 2 else ysent
                        ytsl = slice((tb % 2) * 512, (tb % 2) * 512 + 512)
                        yt = ydst[row][half:half + 64, ytsl]
                        nc.vector.tensor_tensor(
                            yt, pvs[0:64, :], rec[:], ALU.mult)
                        if tb >= 2:
                            nc.sync.dma_start(
                                out=cc_in[hh * 64:(hh + 1) * 64, ytsl],
                                in_=yt)

                def emit_cc(half):
                    nc.gpsimd.collective_compute(
                        "AllReduce", mybir.AluOpType.add,
                        ins=[cc_in[half, :].opt()],
                        outs=[cc_out[half, :].opt()],
                        replica_groups=[[0, 1], [2, 3], [4, 5], [6, 7]],
                    )

                pending = None  # (hp, tb, pvs_e, pvs_o)
                for it in range(16):
                    hp, tb = it // 4, it % 4
                    he, ho = 2 * hp, 2 * hp + 1
                    if True:
                        tsl = slice(tb * 512, (tb + 1) * 512)
                        pvs_e = psPVp.tile([65, 512], F32, tag="pve", name="pve")
                        pvs_o = psPVp.tile([65, 512], F32, tag="pvo", name="pvo")
                        prev = None  # (PT_e, PT_o, s)
                        for s in range(16):
                            ssl = slice(s * 128, (s + 1) * 128)
                            psL_e = psLp.tile([128, 512], F32, tag="Le", name="Le")
                            psL_o = psLp.tile([128, 512], F32, tag="Lo", name="Lo")
                            nc.tensor.matmul(
                                psL_e[:], kT[hp][0:64, ssl],
                                qT[hp][0:64, tsl],
                                start=True, stop=True, tile_position=(0, 0))
                            nc.tensor.matmul(
                                psL_o[:], kT[hp][64:128, ssl],
                                qT[hp][64:128, tsl],
                                start=True, stop=True, tile_position=(64, 0))
                            # drain previous s's PVs while exp(s) runs
                            if prev is not None:
                                PT_pe, PT_po, ps_ = prev
                                nc.tensor.matmul(
                                    pvs_e[:], v_sb[ps_][:, he, :],
                                    PT_pe[:],
                                    start=(ps_ == 0), stop=(ps_ == 15))
                                nc.tensor.matmul(
                                    pvs_o[:], v_sb[ps_][:, ho, :],
                                    PT_po[:].bitcast(BF),
                                    start=(ps_ == 0), stop=(ps_ == 15))
                            PT_e = PTp.tile([128, 512], BF, tag="pte", name="pte")
                            nc.scalar.activation(PT_e[:], psL_e[:], AF.Exp,
                                                 scale=1.0 / 8.0)
                            PT_o = PTp.tile([128, 512], I16, tag="pto", name="pto")
                            if s in (5, 11):
                                nc.scalar.activation(PT_o[:].bitcast(BF),
                                                     psL_o[:], AF.Exp,
                                                     scale=1.0 / 8.0)
                            else:
                                nc.vector.tensor_scalar(
                                    out=PT_o[:], in0=psL_o[:],
                                    scalar1=A16 / 8.0, scalar2=B16,
                                    op0=ALU.mult, op1=ALU.add)
                            prev = (PT_e, PT_o, s)
                        PT_pe, PT_po, ps_ = prev
                        nc.tensor.matmul(
                            pvs_e[:], v_sb[ps_][:, he, :],
                            PT_pe[:], start=False, stop=True)
                        nc.tensor.matmul(
                            pvs_o[:], v_sb[ps_][:, ho, :],
                            PT_po[:].bitcast(BF),
                            start=False, stop=True)
                    # deferred normalize of the previous iteration; its PE
                    # ops land behind this iteration's dense MM block so the
                    # PE never stalls on the ACT denominator copy
                    if pending is not None:
                        normalize(*pending)
                        if pending[0] == 1 and pending[1] == 3:
                            emit_cc(slice(0, 256))
                    pending = (hp, tb, pvs_e, pvs_o)
                normalize(*pending)
                emit_cc(slice(256, 512))
        pQKV.release()

        # ============ Phase D: exchange + residual + LN1 ============
        def layer_norm_T(src, srcb, dst, dstb, lnw, lnb, psp, rowp, sqp, tmpp):
            """Transposed layernorm over partitions (D axis), stats via
            bf16 ones-matmul on srcb (bf16 twin of src); optional bf16
            copy of the output (dstb)."""
            for tbb in range(TLOC // 512):
                sl = slice(tbb * 512, (tbb + 1) * 512)
                psum_s = psp.tile([128, 512], F32, tag="lns", name="lns")
                psum_q = psp.tile([128, 512], F32, tag="lnq", name="lnq")
                for d in range(8):
                    sq = sqp.tile([128, 512], BF, tag="sq", name="sq")
                    nc.vector.tensor_tensor(sq[:], srcb[d][:, sl],
                                            srcb[d][:, sl], ALU.mult)
                    nc.tensor.matmul(psum_s[:], ones_mat[:], srcb[d][:, sl],
                                     start=(d == 0), stop=(d == 7))
                    nc.tensor.matmul(psum_q[:], ones_mat[:], sq[:],
                                     start=(d == 0), stop=(d == 7))
                mean = rowp.tile([128, 512], F32, tag="mean", name="mean")
                nc.vector.tensor_scalar_mul(mean[:], psum_s[:], 1.0 / D)
                m2 = rowp.tile([128, 512], F32, tag="m2", name="m2")
                nc.vector.tensor_tensor(m2[:], mean[:], mean[:], ALU.mult)
                var = rowp.tile([128, 512], F32, tag="var", name="var")
                nc.vector.scalar_tensor_tensor(
                    out=var[:], in0=psum_q[:], scalar=1.0 / D, in1=m2[:],
                    op0=ALU.mult, op1=ALU.subtract)
                std = rowp.tile([128, 512], F32, tag="std", name="std")
                nc.scalar.activation(std[:], var[:], AF.Sqrt, bias=eps128[:])
                rstd = rowp.tile([128, 512], F32, tag="rstd", name="rstd")
                nc.vector.reciprocal_approx_fast(out=rstd[:], in_=std[:])
                ms = rowp.tile([128, 512], F32, tag="ms", name="ms")
                nc.vector.tensor_tensor(ms[:], mean[:], rstd[:], ALU.mult)
                for d in range(8):
                    tmp = tmpp.tile([128, 512], F32, tag="lt", name="lt")
                    nc.vector.tensor_tensor(tmp[:], src[d][:, sl], rstd[:],
                                            ALU.mult)
                    tmp2 = tmpp.tile([128, 512], F32, tag="lt2", name="lt2")
                    nc.vector.tensor_tensor(tmp2[:], tmp[:], ms[:],
                                            ALU.subtract)
                    nc.vector.tensor_scalar(
                        out=dst[d][:, sl], in0=tmp2[:],
                        scalar1=lnw[:, d:d + 1], scalar2=lnb[:, d:d + 1],
                        op0=ALU.mult, op1=ALU.add)
                    if dstb is not None:
                        nc.vector.tensor_scalar(
                            out=dstb[d][:, sl], in0=tmp2[:],
                            scalar1=lnw[:, d:d + 1], scalar2=lnb[:, d:d + 1],
                            op0=ALU.mult, op1=ALU.add)

        pXNB = tc.alloc_tile_pool(name="pXNB", bufs=1)
        xnb = [pXNB.tile([128, TLOC], BF, tag=f"xnb{d}", name=f"xnb{d}")
               for d in range(8)]
        with tc.tile_pool(name="ypart", bufs=1) as ypartp, \
             tc.tile_pool(name="psD", bufs=2, space="PSUM") as psD, \
             tc.tile_pool(name="lnrow", bufs=2) as lnrow, \
             tc.tile_pool(name="lnsq", bufs=3) as lnsq, \
             tc.tile_pool(name="lntmp", bufs=3) as lntmp:
            with nc.named_scope("phD_exch_ln1"):
                yp = [ypartp.tile([128, TLOC], F32, tag=f"yp{i}", name=f"yp{i}")
                      for i in range(4)]
                for r4 in range(4):
                    nc.sync.dma_start(
                        out=yp[r4][:],
                        in_=cc_out[r4 * 128:(r4 + 1) * 128, :])
                for r4 in range(4):
                    nc.gpsimd.tensor_tensor(yp[r4][:], yp[r4][:],
                                            ysent[r4][:], ALU.subtract)
                # residual (rotated D order: chunks 0-3 mine, 4-7 partner)
                for d in range(8):
                    ysrc = ymine[d][:] if d < 4 else yp[d - 4][:]
                    nc.vector.scalar_tensor_tensor(
                        out=xsb[d][:], in0=ysrc, scalar=bv_sb[:, d:d + 1],
                        in1=xm[d][:], op0=ALU.add, op1=ALU.add)
                    nc.vector.scalar_tensor_tensor(
                        out=xm[d][:], in0=ysrc, scalar=bv_sb[:, d:d + 1],
                        in1=xm[d][:], op0=ALU.add, op1=ALU.add)
                layer_norm_T(xm, xsb, xm, xnb, lnw1_sb, lnb1_sb,
                             psD, lnrow, lnsq, lntmp)

        # ================= Phase E: FFN =================
        xn = xm      # LN1 output (f32) in place; xnb is its bf16 copy
        r2 = xm      # FFN residual written back in place
        with tc.tile_pool(name="wff", bufs=2) as wffp, \
             tc.tile_pool(name="wout", bufs=8) as woutp, \
             tc.tile_pool(name="hbuf", bufs=32) as hbufp:
            with nc.named_scope("phE_ffn1"), \
                 tc.tile_pool(name="psH", bufs=4, space="PSUM") as psH:
                h_sb = []
                for blk in range(8):
                    wt = []
                    for d in range(8):
                        w = wffp.tile([128, 512], BF, tag=f"wf{d}", name=f"wf{d}")
                        nc.sync.dma_start(
                            out=w[:],
                            in_=wff_d[d * 128:(d + 1) * 128,
                                      blk * 512:(blk + 1) * 512])
                        wt.append(w)
                    for j in range(4):
                        dt_i = blk * 4 + j
                        h = hbufp.tile([128, TLOC], BF, tag="hb", name="hb")
                        for t2 in range(2):
                            sl = slice(t2 * 512, (t2 + 1) * 512)
                            ps = psH.tile([128, 512], F32, tag="h", name="h")
                            for d in range(8):
                                nc.tensor.matmul(
                                    ps[:], wt[d][:, j * 128:(j + 1) * 128],
                                    xnb[d][:, sl],
                                    start=(d == 0), stop=(d == 7))
                            nc.scalar.activation(
                                h[:, sl], ps[:], AF.Relu,
                                bias=bff_sb[:, dt_i:dt_i + 1])
                        h_sb.append(h)
            with nc.named_scope("phE_ffn2"), \
                 tc.tile_pool(name="psO", bufs=4, space="PSUM") as psO:
                for grp in range(2):
                    pso = [psO.tile([128, TLOC], F32, tag="o", name="o")
                           for _ in range(4)]
                    for c in range(32):
                        wo = woutp.tile([128, 512], BF, tag="wo", name="wo")
                        nc.sync.dma_start(
                            out=wo[:],
                            in_=wout_d[c * 128:(c + 1) * 128,
                                       grp * 512:(grp + 1) * 512])
                        for dt_i in range(4):
                            for t2 in range(2):
                                sl = slice(t2 * 512, (t2 + 1) * 512)
                                nc.tensor.matmul(
                                    pso[dt_i][:, sl],
                                    wo[:, dt_i * 128:(dt_i + 1) * 128],
                                    h_sb[c][:, sl],
                                    start=(c == 0), stop=(c == 31))
                    for dt_i in range(4):
                        dd = grp * 4 + dt_i
                        nc.vector.scalar_tensor_tensor(
                            out=xsb[dd][:], in0=pso[dt_i][:],
                            scalar=bout_sb[:, dd:dd + 1],
                            in1=xn[dd][:], op0=ALU.add, op1=ALU.add)
                        nc.vector.scalar_tensor_tensor(
                            out=r2[dd][:], in0=pso[dt_i][:],
                            scalar=bout_sb[:, dd:dd + 1],
                            in1=xn[dd][:], op0=ALU.add, op1=ALU.add)
        pXNB.release()

        # ============ Phase F: LN2 + store (transposed) ============
        with tc.tile_pool(name="psD2", bufs=2, space="PSUM") as psD2, \
             tc.tile_pool(name="lnrow2", bufs=2) as lnrow2, \
             tc.tile_pool(name="lnsq2", bufs=3) as lnsq2, \
             tc.tile_pool(name="lntmp2", bufs=3) as lntmp2, \
             tc.tile_pool(name="ost", bufs=2) as ostp:
            with nc.named_scope("phF_ln2_out"):
                o32 = [ostp.tile([128, TLOC], F32, tag=f"o{d}", name=f"o{d}")
                       for d in range(8)]
                layer_norm_T(r2, xsb, o32, None, lnw2_sb, lnb2_sb,
                             psD2, lnrow2, lnsq2, lntmp2)
                for d in range(8):
                    nc.sync.dma_start(out=out_d[d * 128:(d + 1) * 128, :],
                                      in_=o32[d][:])
        pY.release()
        pXSB.release()
        pXM.release()
        dramp.release()
        constp.release()

    nc.compile()
    return nc


def _get_program():
    global _PROGRAM
    if _PROGRAM is None:
        _PROGRAM = _build_program()
    return _PROGRAM


def _rotations(hg):
    d0 = hg * 512
    drot = (np.arange(D) + d0) % D
    return d0, drot


def _make_in_maps(x, w_qkv, b_qkv, w_ff, b_ff, w_out, b_out,
                  ln1_w, ln1_b, ln2_w, ln2_b):
    # reference packs qkv interleaved: col(h, dh, sel) = h*192 + dh*3 + sel
    hd = np.arange(H * DH)
    qcols = (hd // DH) * (3 * DH) + (hd % DH) * 3
    kcols = qcols + 1
    vcols = qcols + 2
    in_maps = []
    for c in range(N_CORES):
        b = c // 2
        hg = c % 2
        t0 = hg * TLOC
        d0, drot = _rotations(hg)
        x_rot = np.concatenate([x[b, t0:t0 + TLOC, :],
                                x[b, TLOC - t0:T - t0, :]], axis=0)[:, drot]
        xT = np.ascontiguousarray(x_rot.T)          # [D, T]
        im = {
            "xT": np.ascontiguousarray(xT[:, :TLOC]),
            "xb": np.ascontiguousarray(xT.astype(BF16NP)),
            "wq": np.ascontiguousarray(
                w_qkv[drot][:, qcols[d0:d0 + 512]].astype(BF16NP)),
            "wk": np.ascontiguousarray(
                w_qkv[drot][:, kcols[d0:d0 + 512]].astype(BF16NP)),
            "wv": np.ascontiguousarray(
                w_qkv[drot][:, vcols[d0:d0 + 512]].astype(BF16NP)),
            "bq": np.ascontiguousarray(
                b_qkv[qcols[d0:d0 + 512]].reshape(4, 128).T),
            "bk": np.ascontiguousarray(
                b_qkv[kcols[d0:d0 + 512]].reshape(4, 128).T),
            "bv": np.ascontiguousarray(
                b_qkv[vcols][drot].reshape(8, 128).T),
            "wff": np.ascontiguousarray(w_ff[drot, :].astype(BF16NP)),
            "bff": np.ascontiguousarray(b_ff.reshape(32, 128).T),
            "wout": np.ascontiguousarray(w_out[:, drot].astype(BF16NP)),
            "bout": np.ascontiguousarray(b_out[drot].reshape(8, 128).T),
            "lnw1": np.ascontiguousarray(ln1_w[drot].reshape(8, 128).T),
            "lnb1": np.ascontiguousarray(ln1_b[drot].reshape(8, 128).T),
            "lnw2": np.ascontiguousarray(ln2_w[drot].reshape(8, 128).T),
            "lnb2": np.ascontiguousarray(ln2_b[drot].reshape(8, 128).T),
        }
        in_maps.append(im)
    return in_maps


def _assemble(results):
    out = np.empty((B, T, D), dtype=np.float32)
    for c in range(N_CORES):
        b = c // 2
        hg = c % 2
        _, drot = _rotations(hg)
        inv = np.argsort(drot)
        out[b, hg * TLOC:(hg + 1) * TLOC, :] = results[c]["outT"].T[:, inv]
    return out


def _numpy_fallback(x, mask, w_qkv, b_qkv, w_ff, b_ff, w_out, b_out,
                    ln1_w, ln1_b, ln2_w, ln2_b):
    def ln(v, w, b):
        mu = v.mean(-1, keepdims=True)
        var = ((v - mu) ** 2).mean(-1, keepdims=True)
        return (v - mu) / np.sqrt(var + LN_EPS) * w + b
    b, t, _ = x.shape
    qkv = x @ w_qkv + b_qkv
    qkv = qkv.reshape(b, t, H, DH, 3).transpose(4, 0, 2, 1, 3)
    q, k, v = qkv[0], qkv[1], qkv[2]
    logits = np.einsum("bhtd,bhsd->bhts", q, k) / np.sqrt(DH)
    logits = logits + (1.0 - mask) * -10000.0
    m = logits.max(-1, keepdims=True)
    e = np.exp(logits - m)
    w = e / e.sum(-1, keepdims=True)
    y = np.einsum("bhts,bhsd->bhtd", w, v)
    y = y.transpose(0, 2, 1, 3).reshape(b, t, H * DH)
    x1 = ln(x + y, ln1_w, ln1_b)
    y2 = np.maximum(x1 @ w_ff + b_ff, 0.0) @ w_out + b_out
    return ln(x1 + y2, ln2_w, ln2_b).astype(np.float32)


def kernel(x, mask, w_qkv, b_qkv, w_ff, b_ff, w_out, b_out,
           ln1_w, ln1_b, ln2_w, ln2_b):
    args = [np.ascontiguousarray(np.asarray(a, dtype=np.float32))
            for a in (x, mask, w_qkv, b_qkv, w_ff, b_ff, w_out, b_out,
                      ln1_w, ln1_b, ln2_w, ln2_b)]
    (x, mask, w_qkv, b_qkv, w_ff, b_ff, w_out, b_out,
     ln1_w, ln1_b, ln2_w, ln2_b) = args

    if not np.all(mask == 1.0):
        return _numpy_fallback(x, mask, w_qkv, b_qkv, w_ff, b_ff, w_out, b_out,
                               ln1_w, ln1_b, ln2_w, ln2_b)

    _install_ntff_hook()
    from concourse.bass_utils import run_bass_kernel_spmd

    nc = _get_program()
    in_maps = _make_in_maps(x, w_qkv, b_qkv, w_ff, b_ff, w_out, b_out,
                            ln1_w, ln1_b, ln2_w, ln2_b)

    kw = {}
    if os.environ.get("BASSK_TRACE"):
        kw = dict(trace=True, trace_cores=[0],
                  tmpdir=os.environ.get("BASSK_TRACEDIR", "/tmp/kernel_trace"))
    res = run_bass_kernel_spmd(nc, in_maps, core_ids=list(range(N_CORES)), **kw)
    kernel._last_results = res
    return _assemble(res.results)


# revision 18
# speedup vs baseline: 1.0385x; 1.0385x over previous
"""Trainium2 Bass kernel for a dense transformer decoder layer (fp32 I/O).

Model: B=4, T=2048, H=16 heads, DH=64, D=1024, DFF=4096.
  qkv = x @ w_qkv + b_qkv ; non-causal attention (mask==1) ; residual+LN1 ;
  relu FFN (D->DFF->D) ; residual+LN2.

Sharding over 8 NeuronCores: core c handles batch b=c//2 and head-group
hg=c%2 (8 of 16 heads) for QKV+attention over the full sequence; the core
pair (2b, 2b+1) exchanges attention-output halves with a pairwise
AllReduce(add), and each core runs LN1/FFN/LN2 for its 1024-token half.

v2 design notes (vs the f32r baseline):
  - Host pre-transposes x and pre-packs all weights in bf16; all matmuls
    are bf16 x bf16 (separate, overlappable LDWEIGHTS; f32r is
    self-loading so every MM paid its weight load serially).
  - Attention processes head pairs with row-tiled concurrent matmuls
    (tile_position (0,0)/(64,0)) for the K=64 logits contraction.
  - Softmax exp is split across engines: even head on ACT (Exp, bf16
    out), odd head on DVE via a 1-op Schraudolph in bf16 bit-space
    (tensor_scalar f32->int16, reinterpret as bf16).
  - Softmax denominators ride as a ones-row in the PV stationary; the
    reciprocal uses reciprocal_approx_fast (DVE InstReciprocal is 3.3us).
  - The pairwise exchange is AllReduce(add) on the partner-token half;
    my-half y and my-sent stay in SBUF (no DRAM round trip).
  - FFN weights are loaded once (token-loop inside), h kept in bf16,
    relu+bias on ACT.
"""
import os
import sys
import types

import numpy as np
import ml_dtypes

if "/opt/trn_rl_repo" not in sys.path:
    sys.path.insert(0, "/opt/trn_rl_repo")

BF16NP = ml_dtypes.bfloat16

B, T, H, DH = 4, 2048, 16, 64
D = H * DH            # 1024
DFF = 4096
LN_EPS = 1e-5
N_CORES = 8
TLOC = T // 2         # tokens per core in the FFN phase
HLOC = H // 2         # heads per core

A16 = 184.6650        # 2^7 / ln 2 (bf16 bit-space Schraudolph)
B16 = 16250.0

_PROGRAM = None


def _install_ntff_hook():
    try:
        import antenv
        if "antenv.axon_hooks" in sys.modules:
            return
        mod = types.ModuleType("antenv.axon_hooks")
        holder = [None]
        mod.set_axon_ntff_profile_hook = lambda h: holder.__setitem__(0, h)
        mod.get_axon_ntff_profile_hook = lambda: holder[0]
        sys.modules["antenv.axon_hooks"] = mod
        antenv.axon_hooks = mod
        from trn_agent_boot.trn_boot import _ntff_profile_via_ctypes
        mod.set_axon_ntff_profile_hook(
            _ntff_profile_via_ctypes("/opt/axon/libaxon_pjrt.so"))
    except Exception:
        pass


def _build_program():
    import concourse.bass as bass
    import concourse.mybir as mybir
    import concourse.tile as tile
    from concourse import bacc

    F32 = mybir.dt.float32
    BF = mybir.dt.bfloat16
    I16 = mybir.dt.int16
    AF = mybir.ActivationFunctionType
    ALU = mybir.AluOpType

    nc = bacc.Bacc("TRN2", target_bir_lowering=False, debug=False,
                   num_devices=N_CORES)

    xT_d = nc.dram_tensor("xT", [D, TLOC], F32, kind="ExternalInput").ap()
    xb_d = nc.dram_tensor("xb", [D, T], BF, kind="ExternalInput").ap()
    wq_d = nc.dram_tensor("wq", [D, 512], BF, kind="ExternalInput").ap()
    wk_d = nc.dram_tensor("wk", [D, 512], BF, kind="ExternalInput").ap()
    wv_d = nc.dram_tensor("wv", [D, 512], BF, kind="ExternalInput").ap()
    bq_d = nc.dram_tensor("bq", [128, 4], F32, kind="ExternalInput").ap()
    bk_d = nc.dram_tensor("bk", [128, 4], F32, kind="ExternalInput").ap()
    bv_d = nc.dram_tensor("bv", [128, 8], F32, kind="ExternalInput").ap()
    wff_d = nc.dram_tensor("wff", [D, DFF], BF, kind="ExternalInput").ap()
    bff_d = nc.dram_tensor("bff", [128, 32], F32, kind="ExternalInput").ap()
    wout_d = nc.dram_tensor("wout", [DFF, D], BF, kind="ExternalInput").ap()
    bout_d = nc.dram_tensor("bout", [128, 8], F32, kind="ExternalInput").ap()
    lnw1_d = nc.dram_tensor("lnw1", [128, 8], F32, kind="ExternalInput").ap()
    lnb1_d = nc.dram_tensor("lnb1", [128, 8], F32, kind="ExternalInput").ap()
    lnw2_d = nc.dram_tensor("lnw2", [128, 8], F32, kind="ExternalInput").ap()
    lnb2_d = nc.dram_tensor("lnb2", [128, 8], F32, kind="ExternalInput").ap()
    out_d = nc.dram_tensor("outT", [D, TLOC], F32, kind="ExternalOutput").ap()

    with tile.TileContext(nc) as tc:
        constp = tc.alloc_tile_pool(name="const", bufs=1)
        dramp = tc.alloc_tile_pool(name="dram", bufs=1, space="DRAM")

        eps128 = constp.tile([128, 1], F32)
        nc.vector.memset(eps128[:], LN_EPS)
        ones_mat = constp.tile([128, 128], BF)
        nc.vector.memset(ones_mat[:].bitcast(mybir.dt.uint16), 0x3F80)
        ones_c64 = constp.tile([1, 64], BF)
        nc.vector.memset(ones_c64[:].bitcast(mybir.dt.uint16), 0x3F80)

        bias_tiles = {}
        for name, d_ap, w in [("bq", bq_d, 4), ("bk", bk_d, 4), ("bv", bv_d, 8),
                              ("bff", bff_d, 32), ("bout", bout_d, 8),
                              ("lnw1", lnw1_d, 8), ("lnb1", lnb1_d, 8),
                              ("lnw2", lnw2_d, 8), ("lnb2", lnb2_d, 8)]:
            t = constp.tile([128, w], F32, tag=name)
            nc.sync.dma_start(out=t[:], in_=d_ap)
            bias_tiles[name] = t
        bq_sb, bk_sb, bv_sb = bias_tiles["bq"], bias_tiles["bk"], bias_tiles["bv"]
        bff_sb, bout_sb = bias_tiles["bff"], bias_tiles["bout"]
        lnw1_sb, lnb1_sb = bias_tiles["lnw1"], bias_tiles["lnb1"]
        lnw2_sb, lnb2_sb = bias_tiles["lnw2"], bias_tiles["lnb2"]

        # ============ persistent SBUF state ============
        pXM = tc.alloc_tile_pool(name="pXM", bufs=1)
        xm = [pXM.tile([128, TLOC], F32, tag=f"xm{d}", name=f"xm{d}")
              for d in range(8)]
        pXSB = tc.alloc_tile_pool(name="pXSB", bufs=1)
        xsb = [pXSB.tile([128, TLOC], BF, tag=f"xsb{d}", name=f"xsb{d}")
               for d in range(8)]
        pY = tc.alloc_tile_pool(name="pY", bufs=1)
        ymine = [pY.tile([128, TLOC], F32, tag=f"ym{i}", name=f"ym{i}")
                 for i in range(4)]
        ysent = [pY.tile([128, TLOC], F32, tag=f"ys{i}", name=f"ys{i}")
                 for i in range(4)]
        pQKV = tc.alloc_tile_pool(name="pQKV", bufs=1)
        qT = [pQKV.tile([128, T], BF, tag=f"qT{i}", name=f"qT{i}")
              for i in range(4)]
        kT = [pQKV.tile([128, T], BF, tag=f"kT{i}", name=f"kT{i}")
              for i in range(4)]
        v_sb = [pQKV.tile([128, 8, 65], BF, tag=f"v{i}", name=f"v{i}")
                for i in range(16)]
        cc_in = dramp.tile([512, TLOC], F32, tag="ccin", name="ccin")
        cc_out = dramp.tile([512, TLOC], F32, tag="ccout", name="ccout")

        # ================= Phase B: QKV projections =================
        with tc.tile_pool(name="xbp", bufs=1) as xbp, \
             tc.tile_pool(name="wqk", bufs=1) as wqkp, \
             tc.tile_pool(name="psQK", bufs=6, space="PSUM") as psQK, \
             tc.tile_pool(name="psV", bufs=2, space="PSUM") as psV:
            xb = [xbp.tile([128, T], BF, tag=f"xb{d}", name=f"xb{d}")
                  for d in range(8)]
            for d in range(8):
                nc.sync.dma_start(out=xb[d][:],
                                  in_=xb_d[d * 128:(d + 1) * 128, :])
            with nc.named_scope("phB_qkv"):
                # q/k projections: stationary w chunk, moving xb
                wq_sb = [wqkp.tile([128, 512], BF, tag=f"wq{d}", name=f"wq{d}") for d in range(8)]
                wk_sb = [wqkp.tile([128, 512], BF, tag=f"wk{d}", name=f"wk{d}") for d in range(8)]
                wv_sb = [wqkp.tile([128, 512], BF, tag=f"wv{d}", name=f"wv{d}") for d in range(8)]
                for d in range(8):
                    nc.sync.dma_start(out=wq_sb[d][:],
                                      in_=wq_d[d * 128:(d + 1) * 128, :])
                    nc.sync.dma_start(out=wk_sb[d][:],
                                      in_=wk_d[d * 128:(d + 1) * 128, :])
                    nc.sync.dma_start(out=wv_sb[d][:],
                                      in_=wv_d[d * 128:(d + 1) * 128, :])
                for ct in range(4):
                    for (w_sb, b_sb, dst) in [(wk_sb, bk_sb, kT),
                                              (wq_sb, bq_sb, qT)]:
                        pss = [psQK.tile([128, 512], F32, tag="qk", name="qk")
                               for _ in range(4)]
                        for d in range(8):
                            for tb in range(4):
                                nc.tensor.matmul(
                                    pss[tb][:],
                                    w_sb[d][:, ct * 128:(ct + 1) * 128],
                                    xb[d][:, tb * 512:(tb + 1) * 512],
                                    start=(d == 0), stop=(d == 7))
                        for tb in range(4):
                            nc.vector.tensor_scalar_add(
                                dst[ct][:, tb * 512:(tb + 1) * 512],
                                pss[tb][:], b_sb[:, ct:ct + 1])
                # v projection: stationary xb chunk, moving wv
                for tt in range(16):
                    ps = psV.tile([128, 512], F32, tag="v", name="v")
                    for d in range(8):
                        nc.tensor.matmul(
                            ps[:], xb[d][:, tt * 128:(tt + 1) * 128],
                            wv_sb[d][:], start=(d == 0), stop=(d == 7))
                    nc.vector.tensor_copy(
                        v_sb[tt][:, :, 0:64],
                        ps[:].rearrange("p (h e) -> p h e", h=8))
                    nc.vector.memset(
                        v_sb[tt][:, :, 64:65].bitcast(mybir.dt.uint16), 0x3F80)

        # ================= Phase C: attention =================
        for d in range(8):
            nc.sync.dma_start(out=xm[d][:], in_=xT_d[d * 128:(d + 1) * 128, :])
        with tc.tile_pool(name="psL", bufs=2, space="PSUM") as psLp, \
             tc.tile_pool(name="psPV", bufs=2, space="PSUM") as psPVp, \
             tc.tile_pool(name="PT", bufs=4) as PTp, \
             tc.tile_pool(name="nrm", bufs=4) as nrmp:
            with nc.named_scope("phC_attn"):
                def normalize(hp, tb, pvs_e, pvs_o):
                    he, ho = 2 * hp, 2 * hp + 1
                    for hh, pvs in ((he, pvs_e), (ho, pvs_o)):
                        dbr = nrmp.tile([1, 512], BF, tag="dbr", name="dbr")
                        nc.scalar.copy(dbr[:], pvs[64:65, :])
                        bcp = psLp.tile([128, 512], F32, tag="Le",
                                        name="bcp")
                        nc.tensor.matmul(bcp[0:64, :], ones_c64[:],
                                         dbr[:], start=True, stop=True)
                        rec = nrmp.tile([64, 512], F32, tag="rec", name="rec")
                        nc.vector.reciprocal_approx_fast(
                            out=rec[:], in_=bcp[0:64, :])
                        row = (hh // 2)
                        half = (hh % 2) * 64
                        ydst = ymine if tb < 2 else ysent
                        ytsl = slice((tb % 2) * 512, (tb % 2) * 512 + 512)
                        yt = ydst[row][half:half + 64, ytsl]
                        nc.vector.tensor_tensor(
                            yt, pvs[0:64, :], rec[:], ALU.mult)
                        if tb >= 2:
                            nc.sync.dma_start(
                                out=cc_in[hh * 64:(hh + 1) * 64, ytsl],
                                in_=yt)

                def emit_cc(half):
                    nc.gpsimd.collective_compute(
                        "AllReduce", mybir.AluOpType.add,
                        ins=[cc_in[half, :].opt()],
                        outs=[cc_out[half, :].opt()],
                        replica_groups=[[0, 1], [2, 3], [4, 5], [6, 7]],
                    )

                pending = None  # (hp, tb, pvs_e, pvs_o)
                for it in range(16):
                    hp, tb = it // 4, it % 4
                    he, ho = 2 * hp, 2 * hp + 1
                    if True:
                        tsl = slice(tb * 512, (tb + 1) * 512)
                        pvs_e = psPVp.tile([65, 512], F32, tag="pve", name="pve")
                        pvs_o = psPVp.tile([65, 512], F32, tag="pvo", name="pvo")
                        prev = None  # (PT_e, PT_o, s)
                        for s in range(16):
                            ssl = slice(s * 128, (s + 1) * 128)
                            psL_e = psLp.tile([128, 512], F32, tag="Le", name="Le")
                            psL_o = psLp.tile([128, 512], F32, tag="Lo", name="Lo")
                            nc.tensor.matmul(
                                psL_e[:], kT[hp][0:64, ssl],
                                qT[hp][0:64, tsl],
                                start=True, stop=True, tile_position=(0, 0))
                            nc.tensor.matmul(
                                psL_o[:], kT[hp][64:128, ssl],
                                qT[hp][64:128, tsl],
                                start=True, stop=True, tile_position=(64, 0))
                            # drain previous s's PVs while exp(s) runs
                            if prev is not None:
                                PT_pe, PT_po, ps_ = prev
                                nc.tensor.matmul(
                                    pvs_e[:], v_sb[ps_][:, he, :],
                                    PT_pe[:],
                                    start=(ps_ == 0), stop=(ps_ == 15))
                                nc.tensor.matmul(
                                    pvs_o[:], v_sb[ps_][:, ho, :],
                                    PT_po[:].bitcast(BF),
                                    start=(ps_ == 0), stop=(ps_ == 15))
                            PT_e = PTp.tile([128, 512], BF, tag="pte", name="pte")
                            nc.scalar.activation(PT_e[:], psL_e[:], AF.Exp,
                                                 scale=1.0 / 8.0)
                            PT_o = PTp.tile([128, 512], I16, tag="pto", name="pto")
                            nc.vector.tensor_scalar(
                                out=PT_o[:], in0=psL_o[:],
                                scalar1=A16 / 8.0, scalar2=B16,
                                op0=ALU.mult, op1=ALU.add)
                            prev = (PT_e, PT_o, s)
                        PT_pe, PT_po, ps_ = prev
                        nc.tensor.matmul(
                            pvs_e[:], v_sb[ps_][:, he, :],
                            PT_pe[:], start=False, stop=True)
                        nc.tensor.matmul(
                            pvs_o[:], v_sb[ps_][:, ho, :],
                            PT_po[:].bitcast(BF),
                            start=False, stop=True)
                    # deferred normalize of the previous iteration; its PE
                    # ops land behind this iteration's dense MM block so the
                    # PE never stalls on the ACT denominator copy
                    if pending is not None:
                        normalize(*pending)
                        if pending[0] == 1 and pending[1] == 3:
                            emit_cc(slice(0, 256))
                    pending = (hp, tb, pvs_e, pvs_o)
                normalize(*pending)
                emit_cc(slice(256, 512))
        pQKV.release()

        # ============ Phase D: exchange + residual + LN1 ============
        def layer_norm_T(src, srcb, dst, dstb, lnw, lnb, psp, rowp, sqp, tmpp):
            """Transposed layernorm over partitions (D axis), stats via
            bf16 ones-matmul on srcb (bf16 twin of src); optional bf16
            copy of the output (dstb)."""
            for tbb in range(TLOC // 512):
                sl = slice(tbb * 512, (tbb + 1) * 512)
                psum_s = psp.tile([128, 512], F32, tag="lns", name="lns")
                psum_q = psp.tile([128, 512], F32, tag="lnq", name="lnq")
                for d in range(8):
                    sq = sqp.tile([128, 512], BF, tag="sq", name="sq")
                    nc.vector.tensor_tensor(sq[:], srcb[d][:, sl],
                                            srcb[d][:, sl], ALU.mult)
                    nc.tensor.matmul(psum_s[:], ones_mat[:], srcb[d][:, sl],
                                     start=(d == 0), stop=(d == 7))
                    nc.tensor.matmul(psum_q[:], ones_mat[:], sq[:],
                                     start=(d == 0), stop=(d == 7))
                mean = rowp.tile([128, 512], F32, tag="mean", name="mean")
                nc.vector.tensor_scalar_mul(mean[:], psum_s[:], 1.0 / D)
                m2 = rowp.tile([128, 512], F32, tag="m2", name="m2")
                nc.vector.tensor_tensor(m2[:], mean[:], mean[:], ALU.mult)
                var = rowp.tile([128, 512], F32, tag="var", name="var")
                nc.vector.scalar_tensor_tensor(
                    out=var[:], in0=psum_q[:], scalar=1.0 / D, in1=m2[:],
                    op0=ALU.mult, op1=ALU.subtract)
                std = rowp.tile([128, 512], F32, tag="std", name="std")
                nc.scalar.activation(std[:], var[:], AF.Sqrt, bias=eps128[:])
                rstd = rowp.tile([128, 512], F32, tag="rstd", name="rstd")
                nc.vector.reciprocal_approx_fast(out=rstd[:], in_=std[:])
                ms = rowp.tile([128, 512], F32, tag="ms", name="ms")
                nc.vector.tensor_tensor(ms[:], mean[:], rstd[:], ALU.mult)
                for d in range(8):
                    tmp = tmpp.tile([128, 512], F32, tag="lt", name="lt")
                    nc.vector.tensor_tensor(tmp[:], src[d][:, sl], rstd[:],
                                            ALU.mult)
                    tmp2 = tmpp.tile([128, 512], F32, tag="lt2", name="lt2")
                    nc.vector.tensor_tensor(tmp2[:], tmp[:], ms[:],
                                            ALU.subtract)
                    nc.vector.tensor_scalar(
                        out=dst[d][:, sl], in0=tmp2[:],
                        scalar1=lnw[:, d:d + 1], scalar2=lnb[:, d:d + 1],
                        op0=ALU.mult, op1=ALU.add)
                    if dstb is not None:
                        nc.vector.tensor_scalar(
                            out=dstb[d][:, sl], in0=tmp2[:],
                            scalar1=lnw[:, d:d + 1], scalar2=lnb[:, d:d + 1],
                            op0=ALU.mult, op1=ALU.add)

        pXNB = tc.alloc_tile_pool(name="pXNB", bufs=1)
        xnb = [pXNB.tile([128, TLOC], BF, tag=f"xnb{d}", name=f"xnb{d}")
               for d in range(8)]
        with tc.tile_pool(name="ypart", bufs=1) as ypartp, \
             tc.tile_pool(name="psD", bufs=2, space="PSUM") as psD, \
             tc.tile_pool(name="lnrow", bufs=2) as lnrow, \
             tc.tile_pool(name="lnsq", bufs=3) as lnsq, \
             tc.tile_pool(name="lntmp", bufs=3) as lntmp:
            with nc.named_scope("phD_exch_ln1"):
                yp = [ypartp.tile([128, TLOC], F32, tag=f"yp{i}", name=f"yp{i}")
                      for i in range(4)]
                for r4 in range(4):
                    nc.sync.dma_start(
                        out=yp[r4][:],
                        in_=cc_out[r4 * 128:(r4 + 1) * 128, :])
                for r4 in range(4):
                    nc.vector.tensor_tensor(yp[r4][:], yp[r4][:],
                                            ysent[r4][:], ALU.subtract)
                # residual (rotated D order: chunks 0-3 mine, 4-7 partner)
                for d in range(8):
                    ysrc = ymine[d][:] if d < 4 else yp[d - 4][:]
                    nc.vector.scalar_tensor_tensor(
                        out=xsb[d][:], in0=ysrc, scalar=bv_sb[:, d:d + 1],
                        in1=xm[d][:], op0=ALU.add, op1=ALU.add)
                    nc.vector.scalar_tensor_tensor(
                        out=xm[d][:], in0=ysrc, scalar=bv_sb[:, d:d + 1],
                        in1=xm[d][:], op0=ALU.add, op1=ALU.add)
                layer_norm_T(xm, xsb, xm, xnb, lnw1_sb, lnb1_sb,
                             psD, lnrow, lnsq, lntmp)

        # ================= Phase E: FFN =================
        xn = xm      # LN1 output (f32) in place; xnb is its bf16 copy
        r2 = xm      # FFN residual written back in place
        with tc.tile_pool(name="wff", bufs=2) as wffp, \
             tc.tile_pool(name="wout", bufs=8) as woutp, \
             tc.tile_pool(name="hbuf", bufs=32) as hbufp:
            with nc.named_scope("phE_ffn1"), \
                 tc.tile_pool(name="psH", bufs=4, space="PSUM") as psH:
                h_sb = []
                for blk in range(8):
                    wt = []
                    for d in range(8):
                        w = wffp.tile([128, 512], BF, tag=f"wf{d}", name=f"wf{d}")
                        nc.sync.dma_start(
                            out=w[:],
                            in_=wff_d[d * 128:(d + 1) * 128,
                                      blk * 512:(blk + 1) * 512])
                        wt.append(w)
                    for j in range(4):
                        dt_i = blk * 4 + j
                        h = hbufp.tile([128, TLOC], BF, tag="hb", name="hb")
                        for t2 in range(2):
                            sl = slice(t2 * 512, (t2 + 1) * 512)
                            ps = psH.tile([128, 512], F32, tag="h", name="h")
                            for d in range(8):
                                nc.tensor.matmul(
                                    ps[:], wt[d][:, j * 128:(j + 1) * 128],
                                    xnb[d][:, sl],
                                    start=(d == 0), stop=(d == 7))
                            nc.scalar.activation(
                                h[:, sl], ps[:], AF.Relu,
                                bias=bff_sb[:, dt_i:dt_i + 1])
                        h_sb.append(h)
            with nc.named_scope("phE_ffn2"), \
                 tc.tile_pool(name="psO", bufs=4, space="PSUM") as psO:
                for grp in range(2):
                    pso = [psO.tile([128, TLOC], F32, tag="o", name="o")
                           for _ in range(4)]
                    for c in range(32):
                        wo = woutp.tile([128, 512], BF, tag="wo", name="wo")
                        nc.sync.dma_start(
                            out=wo[:],
                            in_=wout_d[c * 128:(c + 1) * 128,
                                       grp * 512:(grp + 1) * 512])
                        for dt_i in range(4):
                            for t2 in range(2):
                                sl = slice(t2 * 512, (t2 + 1) * 512)
                                nc.tensor.matmul(
                                    pso[dt_i][:, sl],
                                    wo[:, dt_i * 128:(dt_i + 1) * 128],
                                    h_sb[c][:, sl],
                                    start=(c == 0), stop=(c == 31))
                    for dt_i in range(4):
                        dd = grp * 4 + dt_i
                        nc.vector.scalar_tensor_tensor(
                            out=xsb[dd][:], in0=pso[dt_i][:],
                            scalar=bout_sb[:, dd:dd + 1],
                            in1=xn[dd][:], op0=ALU.add, op1=ALU.add)
                        nc.vector.scalar_tensor_tensor(
                            out=r2[dd][:], in0=pso[dt_i][:],
                            scalar=bout_sb[:, dd:dd + 1],
                            in1=xn[dd][:], op0=ALU.add, op1=ALU.add)
        pXNB.release()

        # ============ Phase F: LN2 + store (transposed) ============
        with tc.tile_pool(name="psD2", bufs=2, space="PSUM") as psD2, \
             tc.tile_pool(name="lnrow2", bufs=2) as lnrow2, \
             tc.tile_pool(name="lnsq2", bufs=3) as lnsq2, \
             tc.tile_pool(name="lntmp2", bufs=3) as lntmp2, \
             tc.tile_pool(name="ost", bufs=2) as ostp:
            with nc.named_scope("phF_ln2_out"):
                o32 = [ostp.tile([128, TLOC], F32, tag=f"o{d}", name=f"o{d}")
                       for d in range(8)]
                layer_norm_T(r2, xsb, o32, None, lnw2_sb, lnb2_sb,
                             psD2, lnrow2, lnsq2, lntmp2)
                for d in range(8):
                    nc.sync.dma_start(out=out_d[d * 128:(d + 1) * 128, :],
                                      in_=o32[d][:])
        pY.release()
        pXSB.release()
        pXM.release()
        dramp.release()
        constp.release()

    nc.compile()
    return nc


def _get_program():
    global _PROGRAM
    if _PROGRAM is None:
        _PROGRAM = _build_program()
    return _PROGRAM


def _rotations(hg):
    d0 = hg * 512
    drot = (np.arange(D) + d0) % D
    return d0, drot


def _make_in_maps(x, w_qkv, b_qkv, w_ff, b_ff, w_out, b_out,
                  ln1_w, ln1_b, ln2_w, ln2_b):
    # reference packs qkv interleaved: col(h, dh, sel) = h*192 + dh*3 + sel
    hd = np.arange(H * DH)
    qcols = (hd // DH) * (3 * DH) + (hd % DH) * 3
    kcols = qcols + 1
    vcols = qcols + 2
    in_maps = []
    for c in range(N_CORES):
        b = c // 2
        hg = c % 2
        t0 = hg * TLOC
        d0, drot = _rotations(hg)
        x_rot = np.concatenate([x[b, t0:t0 + TLOC, :],
                                x[b, TLOC - t0:T - t0, :]], axis=0)[:, drot]
        xT = np.ascontiguousarray(x_rot.T)          # [D, T]
        im = {
            "xT": np.ascontiguousarray(xT[:, :TLOC]),
            "xb": np.ascontiguousarray(xT.astype(BF16NP)),
            "wq": np.ascontiguousarray(
                w_qkv[drot][:, qcols[d0:d0 + 512]].astype(BF16NP)),
            "wk": np.ascontiguousarray(
                w_qkv[drot][:, kcols[d0:d0 + 512]].astype(BF16NP)),
            "wv": np.ascontiguousarray(
                w_qkv[drot][:, vcols[d0:d0 + 512]].astype(BF16NP)),
            "bq": np.ascontiguousarray(
                b_qkv[qcols[d0:d0 + 512]].reshape(4, 128).T),
            "bk": np.ascontiguousarray(
                b_qkv[kcols[d0:d0 + 512]].reshape(4, 128).T),
            "bv": np.ascontiguousarray(
                b_qkv[vcols][drot].reshape(8, 128).T),
            "wff": np.ascontiguousarray(w_ff[drot, :].astype(BF16NP)),
            "bff": np.ascontiguousarray(b_ff.reshape(32, 128).T),
            "wout": np.ascontiguousarray(w_out[:, drot].astype(BF16NP)),
            "bout": np.ascontiguousarray(b_out[drot].reshape(8, 128).T),
            "lnw1": np.ascontiguousarray(ln1_w[drot].reshape(8, 128).T),
            "lnb1": np.ascontiguousarray(ln1_b[drot].reshape(8, 128).T),
            "lnw2": np.ascontiguousarray(ln2_w[drot].reshape(8, 128).T),
            "lnb2": np.ascontiguousarray(ln2_b[drot].reshape(8, 128).T),
        }
        in_maps.append(im)
    return in_maps


def _assemble(results):
    out = np.empty((B, T, D), dtype=np.float32)
    for c in range(N_CORES):
        b = c // 2
        hg = c % 2
        _, drot = _rotations(hg)
        inv = np.argsort(drot)
        out[b, hg * TLOC:(hg + 1) * TLOC, :] = results[c]["outT"].T[:, inv]
    return out


def _numpy_fallback(x, mask, w_qkv, b_qkv, w_ff, b_ff, w_out, b_out,
                    ln1_w, ln1_b, ln2_w, ln2_b):
    def ln(v, w, b):
        mu = v.mean(-1, keepdims=True)
        var = ((v - mu) ** 2).mean(-1, keepdims=True)
        return (v - mu) / np.sqrt(var + LN_EPS) * w + b
    b, t, _ = x.shape
    qkv = x @ w_qkv + b_qkv
    qkv = qkv.reshape(b, t, H, DH, 3).transpose(4, 0, 2, 1, 3)
    q, k, v = qkv[0], qkv[1], qkv[2]
    logits = np.einsum("bhtd,bhsd->bhts", q, k) / np.sqrt(DH)
    logits = logits + (1.0 - mask) * -10000.0
    m = logits.max(-1, keepdims=True)
    e = np.exp(logits - m)
    w = e / e.sum(-1, keepdims=True)
    y = np.einsum("bhts,bhsd->bhtd", w, v)
    y = y.transpose(0, 2, 1, 3).reshape(b, t, H * DH)
    x1 = ln(x + y, ln1_w, ln1_b)
    y2 = np.maximum(x1 @ w_ff + b_ff, 0.0) @ w_out + b_out
    return ln(x1 + y2, ln2_w, ln2_b).astype(np.float32)


def kernel(x, mask, w_qkv, b_qkv, w_ff, b_ff, w_out, b_out,
           ln1_w, ln1_b, ln2_w, ln2_b):
    args = [np.ascontiguousarray(np.asarray(a, dtype=np.float32))
            for a in (x, mask, w_qkv, b_qkv, w_ff, b_ff, w_out, b_out,
                      ln1_w, ln1_b, ln2_w, ln2_b)]
    (x, mask, w_qkv, b_qkv, w_ff, b_ff, w_out, b_out,
     ln1_w, ln1_b, ln2_w, ln2_b) = args

    if not np.all(mask == 1.0):
        return _numpy_fallback(x, mask, w_qkv, b_qkv, w_ff, b_ff, w_out, b_out,
                               ln1_w, ln1_b, ln2_w, ln2_b)

    _install_ntff_hook()
    from concourse.bass_utils import run_bass_kernel_spmd

    nc = _get_program()
    in_maps = _make_in_maps(x, w_qkv, b_qkv, w_ff, b_ff, w_out, b_out,
                            ln1_w, ln1_b, ln2_w, ln2_b)

    kw = {}
    if os.environ.get("BASSK_TRACE"):
        kw = dict(trace=True, trace_cores=[0],
                  tmpdir=os.environ.get("BASSK_TRACEDIR", "/tmp/kernel_trace"))
    res = run_bass_kernel_spmd(nc, in_maps, core_ids=list(range(N_CORES)), **kw)
    kernel._last_results = res
    return _assemble(res.results)
